# revision 6
# baseline (speedup 1.0000x reference)
"""Memorizing-Transformer layer on 8 TRN2 NeuronCores (Bass/Tile, SPMD).

Sharding: queries (b, s-half) -> core c = 2b + half (128 query rows each);
memory rows sharded 8 x 8192 for the kNN sims + local top-k; mem_vals
replicated for the gather. Collectives: pair-AllGather of x1 (batch block),
all-8 AllGather of x1 (for sims), all-8 AllToAll of top-k candidates,
pair-AllToAll of partial memory-attention results.

Top-k trick: sims are packed with their 13-bit local column index in the
low mantissa bits (values pairwise distinct per row), selected exactly with
max8/match_replace, and unpacked to indices with bitwise ops.
"""

import numpy as np

import concourse.bass as bass
import concourse.bacc as bacc
import concourse.mybir as mybir
import concourse.tile as tile
from concourse import bass_utils
from concourse.masks import make_identity

F32 = mybir.dt.float32
F32R = mybir.dt.float32r
U32 = mybir.dt.uint32
AL = mybir.AluOpType
AF = mybir.ActivationFunctionType

# problem dims (hardcoded per contract)
B, S, E, H, M, K, FF = 4, 256, 1024, 16, 65536, 32, 4096
HD = E // H          # 64
NCORES = 8
P = 128
LN_EPS = 1e-5
L = 24               # local top-L per memory shard (>= max per-shard need)
NEG = -1e30

# shrinkable knobs for simulator validation
CFG = {
    "MS": M // NCORES,   # memory rows per core (8192)
    "use_gelu": True,
}


def _ln_tail(nc, sb, psum_ap, resid_ap, g_bc, b_bc, out_tile, name):
    """out = LN(psum + resid) * g + b   (rows = partitions, norm over 1024 free)."""
    t = sb.tile([P, E], F32, name=f"{name}_t", tag="lnt")
    s1 = sb.tile([P, 1], F32, name=f"{name}_s1", tag="lns1")
    nc.vector.scalar_tensor_tensor(
        out=t[:], in0=psum_ap, scalar=1.0, in1=resid_ap,
        op0=AL.mult, op1=AL.add, accum_out=s1[:],
    )
    sq = sb.tile([P, E], F32, name=f"{name}_sq", tag="lnsq")
    s2 = sb.tile([P, 1], F32, name=f"{name}_s2", tag="lns2")
    nc.scalar.activation(sq[:], t[:], AF.Square, accum_out=s2[:])
    mean = sb.tile([P, 1], F32, name=f"{name}_mean", tag="lnmean")
    nc.vector.tensor_scalar(out=mean[:], in0=s1[:], scalar1=1.0 / E, scalar2=None, op0=AL.mult)
    var = sb.tile([P, 1], F32, name=f"{name}_var", tag="lnvar")
    nc.vector.tensor_scalar(out=var[:], in0=s2[:], scalar1=1.0 / E, scalar2=None, op0=AL.mult)
    m2 = sb.tile([P, 1], F32, name=f"{name}_m2", tag="lnm2")
    nc.vector.tensor_tensor(out=m2[:], in0=mean[:], in1=mean[:], op=AL.mult)
    nc.vector.tensor_tensor(out=var[:], in0=var[:], in1=m2[:], op=AL.subtract)
    nc.vector.tensor_scalar(out=var[:], in0=var[:], scalar1=LN_EPS, scalar2=None, op0=AL.add)
    sd = sb.tile([P, 1], F32, name=f"{name}_sd", tag="lnsd")
    nc.scalar.activation(sd[:], var[:], AF.Sqrt)
    rstd = sb.tile([P, 1], F32, name=f"{name}_rstd", tag="lnrstd")
    nc.vector.reciprocal(rstd[:], sd[:])
    nmr = sb.tile([P, 1], F32, name=f"{name}_nmr", tag="lnnmr")
    nc.vector.tensor_tensor(out=nmr[:], in0=mean[:], in1=rstd[:], op=AL.mult)
    nc.vector.tensor_scalar(out=nmr[:], in0=nmr[:], scalar1=-1.0, scalar2=None, op0=AL.mult)
    xn = sb.tile([P, E], F32, name=f"{name}_xn", tag="lnxn")
    nc.scalar.activation(xn[:], t[:], AF.Identity, bias=nmr[:, :1], scale=rstd[:, :1])
    nc.vector.tensor_tensor(out=xn[:], in0=xn[:], in1=g_bc, op=AL.mult)
    nc.vector.tensor_tensor(out=out_tile[:], in0=xn[:], in1=b_bc, op=AL.add)
    return t


def build(nc: bass.Bass, cfg=CFG):
    MS = cfg["MS"]
    MT = MS * NCORES           # total memory rows (65536, or shrunk for sim)
    NMC = MS // 512            # m-chunks of 512 per core
    gelu_fn = AF.Gelu if cfg["use_gelu"] else AF.Tanh

    # ---------------- I/O ----------------
    xb_d = nc.dram_tensor("xb", [S, E], F32, kind="ExternalInput").ap()
    xq_d = nc.dram_tensor("xq", [P, E], F32, kind="ExternalInput").ap()
    mkT_d = nc.dram_tensor("mkT", [E, MS], F32R, kind="ExternalInput").ap()
    mv_d = nc.dram_tensor("mv", [MT, E], F32, kind="ExternalInput").ap()
    w_names = ["sa_q", "sa_k", "sa_v", "sa_o", "ma_q", "ma_k", "ma_v", "ma_o"]
    w_d = {n: nc.dram_tensor(f"w_{n}", [E, E], F32R, kind="ExternalInput").ap() for n in w_names}
    b_d = {n: nc.dram_tensor(f"b_{n}", [1, E], F32R, kind="ExternalInput").ap() for n in w_names}
    fc1_d = nc.dram_tensor("w_fc1", [E, FF], F32R, kind="ExternalInput").ap()
    fc2_d = nc.dram_tensor("w_fc2", [FF, E], F32R, kind="ExternalInput").ap()
    bfc1_d = nc.dram_tensor("b_fc1", [1, FF], F32R, kind="ExternalInput").ap()
    bfc2_d = nc.dram_tensor("b_fc2", [1, E], F32R, kind="ExternalInput").ap()
    ln_d = {n: nc.dram_tensor(f"ln_{n}", [1, E], F32, kind="ExternalInput").ap()
            for n in ["1g", "1b", "2g", "2b", "3g", "3b"]}
    cid_d = nc.dram_tensor("coreid", [1, 1], U32, kind="ExternalInput").ap()
    out_d = nc.dram_tensor("out", [P, E], F32, kind="ExternalOutput").ap()

    def wre(ap):       # [E, N] weight -> [p, c, N] chunked for lhsT slices
        return ap.rearrange("(c p) n -> p c n", p=P)

    with tile.TileContext(nc) as tc:
        with (
            tc.tile_pool(name="const", bufs=1) as cst,
            tc.tile_pool(name="persist", bufs=1) as per,
            tc.tile_pool(name="dram", bufs=1, space="DRAM") as dr,
        ):
            # ---------------- constants ----------------
            ident = cst.tile([P, P], F32, name="ident")
            make_identity(nc, ident[:])
            ones_f = cst.tile([P, 16], F32, name="ones_f")
            nc.vector.memset(ones_f[:], 1.0)
            ones_fr = cst.tile([1, P], F32, name="ones_fr")
            nc.vector.memset(ones_fr[:], 1.0)
            ones_r1 = cst.tile([1, P], F32R, name="ones_r1")   # rank-1 bias lhsT
            nc.scalar.copy(ones_r1[:], ones_fr[:])
            ones_col = cst.tile([P, 1], F32R, name="ones_col")  # norm lhsT
            nc.scalar.copy(ones_col[:], ones_f[:, :1])
            ones_c2 = cst.tile([P, 2], F32R, name="ones_c2")    # den rhs (even N)
            nc.scalar.copy(ones_c2[:], ones_f[:, :2])
            ones32 = cst.tile([P, 32], F32, name="ones32")
            nc.vector.memset(ones32[:], 1.0)
            mask_hi = cst.tile([P, 1], U32, name="mask_hi")
            nc.vector.memset(mask_hi[:], 0xFFFFE000)
            mask_lo = cst.tile([P, 1], U32, name="mask_lo")
            nc.vector.memset(mask_lo[:], 0x1FFF)
            cid_sb = cst.tile([1, 1], U32, name="cid_sb")
            nc.sync.dma_start(cid_sb[:], cid_d[:])
            cid_bc = cst.tile([P, 1], U32, name="cid_bc")
            nc.gpsimd.partition_broadcast(cid_bc[:], cid_sb[:])
            c8192 = cst.tile([P, 1], U32, name="c8192")
            nc.vector.memset(c8192[:], MS)
            pidsh = cst.tile([P, 1], U32, name="pidsh")     # coreid * MS
            nc.vector.tensor_tensor(out=pidsh[:], in0=cid_bc[:], in1=c8192[:], op=AL.mult)

            # biases as [P, E/P] per-partition tiles (for T-form ACT eviction)
            bpp = {}
            for n in ["sa_q", "sa_k", "ma_q", "ma_k"]:
                bpp[n] = cst.tile([P, E // P], F32, name=f"bpp_{n}")
                nc.sync.dma_start(bpp[n][:], b_d[n].bitcast(F32).rearrange("a (c p) -> (a p) c", p=P))
            # bias rows for rank-1 matmuls
            brow = {}
            for n in ["sa_v", "sa_o", "ma_v", "ma_o"]:
                brow[n] = cst.tile([1, E], F32R, name=f"brow_{n}")
                nc.sync.dma_start(brow[n][:], b_d[n][:])

            # persistent activations
            x1_own = per.tile([P, E], F32, name="x1_own")
            QTma = per.tile([P, E // P, S], F32R, name="QTma")
            idxu = per.tile([P, K], U32, name="idxu")
            oma = per.tile([P, E], F32, name="oma")

            # DRAM bounces for collectives
            x1_bounce = dr.tile([P, E], F32, name="x1_bounce")
            x1_batch_d = dr.tile([S, E], F32, name="x1_batch_d")
            x1_all_d = dr.tile([B * S, E], F32, name="x1_all_d", addr_space="Shared")
            cand_in = dr.tile([B * S, 2 * L], F32, name="cand_in")
            cand_out = dr.tile([B * S, 2 * L], F32, name="cand_out")
            ma_in = dr.tile([S, 1056], F32, name="ma_in")
            ma_out = dr.tile([P, 1056], F32, name="ma_out")

            EC = E // P  # 8 E-chunks

            def proj_T(dst, src_T, w_sb, bias_pp, ncols, psp, extra=None):
                """dst[p, et, ncols] (F32R) = (W^T x)^T i.e. out[e, col]; bias per-e."""
                for et in range(EC):
                    ps = psp.tile([P, 512], F32, name=f"pT_{dst.name}_{et}", tag="pT")
                    for ec in range(EC):
                        nc.tensor.matmul(
                            out=ps[:, :ncols],
                            lhsT=w_sb[:, ec, et * P:(et + 1) * P],
                            rhs=src_T[:, ec, :ncols],
                            start=(ec == 0), stop=(ec == EC - 1),
                        )
                    nc.scalar.activation(dst[:, et, :ncols], ps[:, :ncols], AF.Identity,
                                         bias=bias_pp[:, et:et + 1])

            def transpose_in(dst, src_ap, j, psp, n=P):
                """dst[:, j-slice] (F32R) = src_ap^T for one [P, P] block."""
                ps = psp.tile([P, 256], F32, name=f"tp_{dst.name}_{j}", tag="pT")
                nc.tensor.transpose(out=ps[:, :n], in_=src_ap, identity=ident[:])
                nc.scalar.copy(dst, ps[:, :n])

            # ============ PHASE B: self-attention + LN1 -> x1_own ============
            with (
                tc.tile_pool(name="sbB", bufs=1) as sbB,
                tc.tile_pool(name="wB", bufs=2) as wB,
                tc.tile_pool(name="psB", bufs=2, space="PSUM") as psB,
                tc.tile_pool(name="psOB", bufs=1, space="PSUM") as psOB,
            ):
                def load_w(n):
                    t = wB.tile([P, EC, E], F32R, name=f"w_{n}_sb", tag="wsa")
                    nc.sync.dma_start(t[:], wre(w_d[n]))
                    return t

                lng = sbB.tile([1, E], F32, name="ln1g_row")
                nc.sync.dma_start(lng[:], ln_d["1g"][:])
                lnb = sbB.tile([1, E], F32, name="ln1b_row")
                nc.sync.dma_start(lnb[:], ln_d["1b"][:])
                ln1g_bc = sbB.tile([P, E], F32, name="ln1g_bc")
                nc.gpsimd.partition_broadcast(ln1g_bc[:], lng[:])
                ln1b_bc = sbB.tile([P, E], F32, name="ln1b_bc")
                nc.gpsimd.partition_broadcast(ln1b_bc[:], lnb[:])

                xb0 = sbB.tile([P, E], F32, name="xb0")
                nc.sync.dma_start(xb0[:], xb_d[0:P, :])
                xb1 = sbB.tile([P, E], F32, name="xb1")
                nc.sync.dma_start(xb1[:], xb_d[P:S, :])
                xq = sbB.tile([P, E], F32, name="xq_sb")
                nc.sync.dma_start(xq[:], xq_d[:])

                xbT = sbB.tile([P, EC, S], F32R, name="xbT")
                for sc, xt in enumerate([xb0, xb1]):
                    for ec in range(EC):
                        transpose_in(xbT[:, ec, sc * P:(sc + 1) * P],
                                     xt[:, ec * P:(ec + 1) * P], f"{sc}_{ec}", psB)
                xqT = sbB.tile([P, EC, P], F32R, name="xqT")
                for ec in range(EC):
                    transpose_in(xqT[:, ec, :], xq[:, ec * P:(ec + 1) * P], f"q_{ec}", psB)

                w_cur = load_w("sa_k")
                KTsa = sbB.tile([P, EC, S], F32R, name="KTsa")
                proj_T(KTsa, xbT, w_cur, bpp["sa_k"], S, psB)
                w_cur = load_w("sa_q")
                QTsa = sbB.tile([P, EC, P], F32R, name="QTsa")
                proj_T(QTsa, xqT, w_cur, bpp["sa_q"], P, psB)

                w_cur = load_w("sa_v")
                Vsa = sbB.tile([P, 2, E], F32R, name="Vsa")
                for kt in range(2):
                    for n2 in range(2):
                        ps = psB.tile([P, 512], F32, name=f"pVsa_{kt}_{n2}", tag="pT")
                        for ec in range(EC):
                            nc.tensor.matmul(
                                out=ps[:],
                                lhsT=xbT[:, ec, kt * P:(kt + 1) * P],
                                rhs=w_cur[:, ec, n2 * 512:(n2 + 1) * 512],
                                start=(ec == 0), stop=False,
                            )
                        nc.tensor.matmul(out=ps[:], lhsT=ones_r1[:],
                                         rhs=brow["sa_v"][:, n2 * 512:(n2 + 1) * 512],
                                         start=False, stop=True)
                        nc.scalar.copy(Vsa[:, kt, n2 * 512:(n2 + 1) * 512], ps[:])
                w_sao = load_w("sa_o")

                attnTsa = sbB.tile([P, 2, H, P], F32R, name="attnTsa")
                for kt in range(2):
                    for hd in range(H):
                        r0 = (hd % 2) * HD
                        ps = psB.tile([P, 256], F32, name=f"psc_{kt}_{hd}", tag="pT")
                        nc.tensor.matmul(
                            out=ps[:, :P],
                            lhsT=KTsa[r0:r0 + HD, hd // 2, kt * P:(kt + 1) * P],
                            rhs=QTsa[r0:r0 + HD, hd // 2, :],
                            start=True, stop=True,
                        )
                        nc.scalar.activation(attnTsa[:, kt, hd, :], ps[:, :P], AF.Exp,
                                             scale=float(1.0 / np.sqrt(HD)))
                den_sa = sbB.tile([P, H], F32, name="den_sa")
                for hd in range(H):
                    dt_ = psB.tile([P, 2], F32, name=f"densa_{hd}", tag="densa", bufs=2)
                    for kt in range(2):
                        nc.tensor.matmul(out=dt_[:], lhsT=attnTsa[:, kt, hd, :],
                                         rhs=ones_c2[:], start=(kt == 0), stop=(kt == 1))
                    nc.scalar.copy(den_sa[:, hd:hd + 1], dt_[:, :1])
                rden = sbB.tile([P, H], F32, name="rden_sa")
                nc.vector.reciprocal(rden[:], den_sa[:])
                osa = sbB.tile([P, E], F32, name="osa")
                for hd in range(H):
                    ot = psB.tile([P, HD], F32, name=f"osap_{hd}", tag="osap", bufs=2)
                    for kt in range(2):
                        nc.tensor.matmul(
                            out=ot[:],
                            lhsT=attnTsa[:, kt, hd, :],
                            rhs=Vsa[:, kt, hd * HD:(hd + 1) * HD],
                            start=(kt == 0), stop=(kt == 1),
                        )
                    nc.vector.tensor_scalar(
                        out=osa[:, hd * HD:(hd + 1) * HD], in0=ot[:],
                        scalar1=rden[:, hd:hd + 1], scalar2=None, op0=AL.mult)
                oTsa = sbB.tile([P, EC, P], F32R, name="oTsa")
                for ec in range(EC):
                    transpose_in(oTsa[:, ec, :], osa[:, ec * P:(ec + 1) * P], f"o_{ec}", psB)
                m_sb = sbB.tile([P, E], F32, name="m_sa_sb")
                for n2 in range(2):
                    mp = psOB.tile([P, 512], F32, name=f"m_sa_ps{n2}", tag="msa", bufs=2)
                    for ec in range(EC):
                        nc.tensor.matmul(
                            out=mp[:],
                            lhsT=oTsa[:, ec, :],
                            rhs=w_sao[:, ec, n2 * 512:(n2 + 1) * 512],
                            start=(ec == 0), stop=False)
                    nc.tensor.matmul(out=mp[:], lhsT=ones_r1[:],
                                     rhs=brow["sa_o"][:, n2 * 512:(n2 + 1) * 512],
                                     start=False, stop=True)
                    nc.scalar.copy(m_sb[:, n2 * 512:(n2 + 1) * 512], mp[:])
                _ln_tail(nc, sbB, m_sb[:], xq[:], ln1g_bc[:], ln1b_bc[:], x1_own, "ln1")
                nc.sync.dma_start(x1_bounce[:], x1_own[:])

            # mem-key chunk prefetch: fills the DMA engines during the
            # two x1 collectives below (phase D consumes these first)
            _mkpre = tc.tile_pool(name="mkPre", bufs=1)
            mkPre = _mkpre.__enter__()
            mk_pre = []
            for mc in range(min(4, NMC)):
                t = mkPre.tile([P, EC, 512], F32R, name=f"mkp_{mc}")
                nc.sync.dma_start(
                    t[:], mkT_d[:, mc * 512:(mc + 1) * 512].rearrange("(c p) m -> p c m", p=P))
                mk_pre.append(t)

            # collectives: pair + all-8 gather of x1
            nc.gpsimd.collective_compute(
                "AllGather", AL.bypass, replica_groups=[[0, 1], [2, 3], [4, 5], [6, 7]],
                ins=[x1_bounce[:].opt()], outs=[x1_batch_d[:].opt()])
            nc.gpsimd.collective_compute(
                "AllGather", AL.bypass, replica_groups=[list(range(NCORES))],
                ins=[x1_bounce[:].opt()], outs=[x1_all_d[:].opt()])

            # ============ PHASE C: x1T, x1bT, QTma ============
            _x1cm = tc.tile_pool(name="x1Tp", bufs=1)
            x1p = _x1cm.__enter__()
            x1T = x1p.tile([P, E // P, S * B], F32R, name="x1T")    # all 1024 queries
            x1bT = x1p.tile([P, E // P, S], F32R, name="x1bT")      # own batch 256
            with (
                tc.tile_pool(name="sbC", bufs=2) as sbC,
                tc.tile_pool(name="wC", bufs=1) as wC,
                tc.tile_pool(name="psC", bufs=2, space="PSUM") as psC,
            ):
                for qt in range(B * S // P):
                    x1q = sbC.tile([P, E], F32, name="x1q", tag="x1q")
                    nc.sync.dma_start(x1q[:], x1_all_d[qt * P:(qt + 1) * P, :])
                    for ec in range(EC):
                        transpose_in(x1T[:, ec, qt * P:(qt + 1) * P],
                                     x1q[:, ec * P:(ec + 1) * P], f"{qt}_{ec}", psC)
                for sc in range(2):
                    x1b = sbC.tile([P, E], F32, name="x1b", tag="x1q")
                    nc.sync.dma_start(x1b[:], x1_batch_d[sc * P:(sc + 1) * P, :])
                    for ec in range(EC):
                        transpose_in(x1bT[:, ec, sc * P:(sc + 1) * P],
                                     x1b[:, ec * P:(ec + 1) * P], f"b{sc}_{ec}", psC)
                w_maq = wC.tile([P, EC, E], F32R, name="w_maq_sb")
                nc.sync.dma_start(w_maq[:], wre(w_d["ma_q"]))
                proj_T(QTma, x1bT, w_maq, bpp["ma_q"], S, psC)

            # ============ PHASE D: sims + local top-L ============
            with (
                tc.tile_pool(name="sbD", bufs=2) as sbD,
                tc.tile_pool(name="cbD", bufs=4) as cbD,
                tc.tile_pool(name="CqD", bufs=1) as CqD,
                tc.tile_pool(name="psS", bufs=4, space="PSUM") as psS,
                tc.tile_pool(name="psN", bufs=2, space="PSUM") as psN,
            ):
                NQT = B * S // P
                Cq = [CqD.tile([P, NMC * 8], F32, name=f"Cq_{qt}") for qt in range(NQT)]
                for mc in range(NMC):
                    if mc < len(mk_pre):
                        mknc = mk_pre[mc]
                    else:
                        mknc = sbD.tile([P, EC, 512], F32R, name="mknc", tag="mknc")
                        nc.sync.dma_start(
                            mknc[:], mkT_d[:, mc * 512:(mc + 1) * 512].rearrange("(c p) m -> p c m", p=P))
                    iot = sbD.tile([P, 512], U32, name="iot", tag="iot")
                    nc.gpsimd.iota(iot[:], pattern=[[1, 512]], base=mc * 512, channel_multiplier=0)
                    for qt in range(NQT):
                        ps = psS.tile([P, 512], F32, name=f"psim_{mc}_{qt}", tag="psim")
                        for ec in range(EC):
                            nc.tensor.matmul(out=ps[:], lhsT=x1T[:, ec, qt * P:(qt + 1) * P],
                                             rhs=mknc[:, ec, :],
                                             start=(ec == 0), stop=(ec == EC - 1))
                        cb = cbD.tile([P, 512], F32, name="cb", tag="cb")
                        nc.scalar.copy(cb[:], ps[:])
                        cbu = cbD.tile([P, 512], U32, name="cbu", tag="cbu")
                        nc.vector.scalar_tensor_tensor(
                            out=cbu[:], in0=cb[:].bitcast(U32), scalar=mask_hi[:, :1],
                            in1=iot[:], op0=AL.bitwise_and, op1=AL.bitwise_or)
                        cs = mc * 8
                        nc.vector.max(out=Cq[qt][:, cs:cs + 8], in_=cbu[:].bitcast(F32))

                # local top-L merge + candidate emit
                for qt in range(NQT):
                    lv = sbD.tile([P, L], F32, name="lv", tag="lv")
                    for r in range(L // 8):
                        nc.vector.max(out=lv[:, r * 8:r * 8 + 8], in_=Cq[qt][:])
                        if r < L // 8 - 1:
                            nc.vector.match_replace(out=Cq[qt][:], in_to_replace=lv[:, r * 8:r * 8 + 8],
                                                    in_values=Cq[qt][:], imm_value=NEG)
                    gidx = sbD.tile([P, L], U32, name="gidx", tag="gidx")
                    nc.vector.scalar_tensor_tensor(
                        out=gidx[:], in0=lv[:].bitcast(U32), scalar=mask_lo[:, :1],
                        in1=pidsh[:, :1].to_broadcast([P, L]), op0=AL.bitwise_and,
                        op1=AL.bitwise_or)
                    nc.sync.dma_start(cand_in[qt * P:(qt + 1) * P, 0:L], lv[:])
                    nc.sync.dma_start(cand_in[qt * P:(qt + 1) * P, L:2 * L], gidx[:].bitcast(F32))

            _x1cm.__exit__(None, None, None)
            _mkpre.__exit__(None, None, None)

            nc.gpsimd.collective_compute(
                "AllToAll", AL.bypass, replica_groups=[list(range(NCORES))],
                ins=[cand_in[:].opt()], outs=[cand_out[:].opt()])

            # ============ PHASE E: global top-32 for own queries ============
            with tc.tile_pool(name="sbE", bufs=1) as sbE:
                ca = cand_out[:].rearrange("(c q) l -> c q l", c=NCORES)
                CV3 = sbE.tile([P, NCORES, L], F32, name="CV3")
                nc.sync.dma_start(CV3[:], ca[:, :, 0:L].rearrange("c q l -> q c l"))
                CIu3 = sbE.tile([P, NCORES, L], U32, name="CIu3")
                nc.sync.dma_start(CIu3[:], ca[:, :, L:2 * L].rearrange("c q l -> q c l").bitcast(U32))
                CV = CV3[:].rearrange("q c l -> q (c l)")
                CIu = CIu3[:].rearrange("q c l -> q (c l)")
                CIf = sbE.tile([P, NCORES * L], F32, name="CIf")
                nc.vector.tensor_copy(CIf[:], CIu)
                w8 = sbE.tile([P, 8], F32, name="w8")
                eqm = sbE.tile([P, NCORES * L], F32, name="eqm")
                scr = sbE.tile([P, NCORES * L], F32, name="scr")
                idxf = sbE.tile([P, K], F32, name="idxf")
                for r in range(K // 8):
                    nc.vector.max(out=w8[:], in_=CV)
                    for k2 in range(8):
                        nc.vector.tensor_scalar(out=eqm[:], in0=CV, scalar1=w8[:, k2:k2 + 1],
                                                scalar2=None, op0=AL.is_equal)
                        nc.vector.scalar_tensor_tensor(
                            out=scr[:], in0=eqm[:], scalar=1.0, in1=CIf[:],
                            op0=AL.mult, op1=AL.mult,
                            accum_out=idxf[:, r * 8 + k2:r * 8 + k2 + 1])
                    if r < K // 8 - 1:
                        nc.vector.match_replace(out=CV, in_to_replace=w8[:],
                                                in_values=CV, imm_value=NEG)
                nc.vector.tensor_scalar(out=idxf[:], in0=idxf[:], scalar1=float(MT - 1),
                                        scalar2=0.0, op0=AL.min, op1=AL.max)
                nc.vector.tensor_copy(idxu[:], idxf[:])

            # ============ PHASE F: memory attention ============
            NKC = 16              # chunks of 256 keys
            with (
                tc.tile_pool(name="gF", bufs=2) as gF,
                tc.tile_pool(name="sbF", bufs=1) as sbF,
                tc.tile_pool(name="atF", bufs=1) as atF,
                tc.tile_pool(name="wF", bufs=1) as wF,
                tc.tile_pool(name="psKV", bufs=2, space="PSUM") as psKV,
                tc.tile_pool(name="psAV", bufs=4, space="PSUM") as psAV,
            ):
                w_mak = wF.tile([P, EC, E], F32R, name="w_mak_sb")
                nc.sync.dma_start(w_mak[:], wre(w_d["ma_k"]))
                w_mav = wF.tile([P, EC, E], F32R, name="w_mav_sb")
                nc.sync.dma_start(w_mav[:], wre(w_d["ma_v"]))
                # SBUF accumulator for AV partials: [qh][16 heads x (64 num + 2 den)]
                o_acc = sbF.tile([P, 2, 1056], F32, name="o_acc")
                nc.vector.memset(o_acc[:], 0.0)

                for cj in range(NKC):
                    gbuf = gF.tile([P, 2, E], F32, name="gbuf", tag="gbuf")
                    for j2 in range(2):
                        nc.gpsimd.indirect_dma_start(
                            out=gbuf[:, j2, :], out_offset=None, in_=mv_d[:],
                            in_offset=bass.IndirectOffsetOnAxis(
                                ap=idxu[:, 2 * cj + j2:2 * cj + j2 + 1], axis=0))
                    rT = gF.tile([P, EC, 256], F32R, name="rT", tag="rT")
                    for j2 in range(2):
                        for ec in range(EC):
                            transpose_in(rT[:, ec, j2 * P:(j2 + 1) * P],
                                         gbuf[:, j2, ec * P:(ec + 1) * P], f"g{cj}_{j2}_{ec}", psKV)
                    KTc = gF.tile([P, EC, 256], F32R, name="KTc", tag="KTc")
                    proj_T(KTc, rT, w_mak, bpp["ma_k"], 256, psKV)
                    Vc = gF.tile([P, 2, 1056], F32R, name="Vc", tag="Vc")
                    for j2 in range(2):
                        ones_dst = Vc[:, j2, :].rearrange("p (h t) -> p h t", t=66)[:, :, 64:66]
                        nc.scalar.copy(ones_dst, ones32[:].rearrange("p (h t) -> p h t", t=2))
                    for j2 in range(2):
                        for n2 in range(2):
                            ps = psKV.tile([P, 512], F32, name=f"pv_{cj}_{j2}_{n2}", tag="pT")
                            for ec in range(EC):
                                nc.tensor.matmul(
                                    out=ps[:],
                                    lhsT=rT[:, ec, j2 * P:(j2 + 1) * P],
                                    rhs=w_mav[:, ec, n2 * 512:(n2 + 1) * 512],
                                    start=(ec == 0), stop=False)
                            nc.tensor.matmul(out=ps[:], lhsT=ones_r1[:],
                                             rhs=brow["ma_v"][:, n2 * 512:(n2 + 1) * 512],
                                             start=False, stop=True)
                            for h8 in range(8):
                                hd = n2 * 8 + h8
                                nc.scalar.copy(Vc[:, j2, hd * 66:hd * 66 + 64],
                                               ps[:, h8 * 64:h8 * 64 + 64])
                    for j2 in range(2):
                        attc = atF.tile([P, H, 256], F32R, name="attc", tag="attc")
                        for hd in range(H):
                            r0 = (hd % 2) * HD
                            ps = psKV.tile([P, 512], F32, name=f"psc_{cj}_{j2}_{hd}", tag="pT")
                            nc.tensor.matmul(
                                out=ps[:, :256],
                                lhsT=KTc[r0:r0 + HD, hd // 2, j2 * P:(j2 + 1) * P],
                                rhs=QTma[r0:r0 + HD, hd // 2, :],
                                start=True, stop=True)
                            nc.scalar.activation(attc[:, hd, :], ps[:, :256], AF.Exp,
                                                 scale=float(1.0 / np.sqrt(HD)))
                        for hd in range(H):
                            for qh in range(2):
                                avt = psAV.tile([P, 66], F32, name=f"av_{cj}_{j2}_{hd}_{qh}",
                                                tag="av")
                                nc.tensor.matmul(
                                    out=avt[:],
                                    lhsT=attc[:, hd, qh * P:(qh + 1) * P],
                                    rhs=Vc[:, j2, hd * 66:hd * 66 + 66],
                                    start=True, stop=True)
                                nc.vector.tensor_tensor(
                                    out=o_acc[:, qh, hd * 66:hd * 66 + 66],
                                    in0=o_acc[:, qh, hd * 66:hd * 66 + 66],
                                    in1=avt[:], op=AL.add)
                # exchange partials within pair (ReduceScatter over pairs)
                for qh in range(2):
                    nc.sync.dma_start(ma_in[qh * P:(qh + 1) * P, :], o_acc[:, qh, :])
                nc.gpsimd.collective_compute(
                    "ReduceScatter", AL.add, replica_groups=[[0, 1], [2, 3], [4, 5], [6, 7]],
                    ins=[ma_in[:].opt()], outs=[ma_out[:].opt()])
                tot = sbF.tile([P, 1056], F32, name="tot")
                nc.sync.dma_start(tot[:], ma_out[:])
                den16 = sbF.tile([P, H], F32, name="den16")
                nc.vector.tensor_copy(
                    den16[:], tot[:].rearrange("p (h t) -> p h t", t=66)[:, :, 64])
                rden16 = sbF.tile([P, H], F32, name="rden16")
                nc.vector.reciprocal(rden16[:], den16[:])
                for hd in range(H):
                    nc.vector.tensor_scalar(
                        out=oma[:, hd * HD:(hd + 1) * HD], in0=tot[:, hd * 66:hd * 66 + 64],
                        scalar1=rden16[:, hd:hd + 1], scalar2=None, op0=AL.mult)

            # ============ PHASE G: O-proj + LN2 + FFN + LN3 ============
            with (
                tc.tile_pool(name="sbG", bufs=1) as sbG,
                tc.tile_pool(name="fcG", bufs=3) as fcG,
                tc.tile_pool(name="wG", bufs=1) as wG,
                tc.tile_pool(name="psG", bufs=2, space="PSUM") as psG,
                tc.tile_pool(name="psM", bufs=1, space="PSUM") as psM,
            ):
                oTma = sbG.tile([P, EC, P], F32R, name="oTma", tag="octT")
                for ec in range(EC):
                    transpose_in(oTma[:, ec, :], oma[:, ec * P:(ec + 1) * P], f"om_{ec}", psG)
                w_mao = wG.tile([P, EC, E], F32R, name="w_mao_sb")
                nc.sync.dma_start(w_mao[:], wre(w_d["ma_o"]))
                m_sb = sbG.tile([P, E], F32, name="m_ma_sb")
                for n2 in range(2):
                    mp = psM.tile([P, 512], F32, name=f"m_ma_ps{n2}", tag="mma", bufs=2)
                    for ec in range(EC):
                        nc.tensor.matmul(
                            out=mp[:], lhsT=oTma[:, ec, :],
                            rhs=w_mao[:, ec, n2 * 512:(n2 + 1) * 512],
                            start=(ec == 0), stop=False)
                    nc.tensor.matmul(out=mp[:], lhsT=ones_r1[:],
                                     rhs=brow["ma_o"][:, n2 * 512:(n2 + 1) * 512],
                                     start=False, stop=True)
                    nc.scalar.copy(m_sb[:, n2 * 512:(n2 + 1) * 512], mp[:])
                brow_fc1 = sbG.tile([1, FF], F32R, name="brow_fc1")
                nc.sync.dma_start(brow_fc1[:], bfc1_d[:])
                brow_fc2 = sbG.tile([1, E], F32R, name="brow_fc2")
                nc.sync.dma_start(brow_fc2[:], bfc2_d[:])
                def ln_bc(which):
                    row = sbG.tile([1, E], F32, name=f"l{which}_row", tag="lnrow", bufs=1)
                    nc.sync.dma_start(row[:], ln_d[which][:])
                    bc = sbG.tile([P, E], F32, name=f"l{which}_bc", tag=f"lnbc{which[1]}", bufs=1)
                    nc.gpsimd.partition_broadcast(bc[:], row[:])
                    return bc

                l2g_bc = ln_bc("2g")
                l2b_bc = ln_bc("2b")

                x2 = sbG.tile([P, E], F32, name="x2")
                _ln_tail(nc, sbG, m_sb[:], x1_own[:], l2g_bc[:], l2b_bc[:], x2, "ln2")
                x2T = sbG.tile([P, EC, P], F32R, name="x2T", tag="octT")
                for ec in range(EC):
                    transpose_in(x2T[:, ec, :], x2[:, ec * P:(ec + 1) * P], f"x2_{ec}", psG)

                g_sb = sbG.tile([P, FF], F32, name="g_sb")
                for n8 in range(FF // 512):
                    fc1c = fcG.tile([P, EC, 512], F32R, name="fc1c", tag="fc1c", bufs=2)
                    nc.sync.dma_start(
                        fc1c[:], fc1_d[:, n8 * 512:(n8 + 1) * 512].rearrange("(c p) n -> p c n", p=P))
                    ps = psG.tile([P, 512], F32, name=f"ph_{n8}", tag="pT")
                    for ec in range(EC):
                        nc.tensor.matmul(out=ps[:], lhsT=x2T[:, ec, :], rhs=fc1c[:, ec, :],
                                         start=(ec == 0), stop=False)
                    nc.tensor.matmul(out=ps[:], lhsT=ones_r1[:],
                                     rhs=brow_fc1[:, n8 * 512:(n8 + 1) * 512],
                                     start=False, stop=True)
                    nc.scalar.activation(g_sb[:, n8 * 512:(n8 + 1) * 512], ps[:], gelu_fn)
                gT = sbG.tile([P, FF // P, P], F32R, name="gT")
                for kc in range(FF // P):
                    transpose_in(gT[:, kc, :], g_sb[:, kc * P:(kc + 1) * P], f"gt_{kc}", psG)
                y_sb = sbG.tile([P, E], F32, name="y_sb")
                for n2 in range(2):
                    yp = psM.tile([P, 512], F32, name=f"y_ps{n2}", tag="mma", bufs=2)
                    for kc in range(FF // P):
                        fc2c = fcG.tile([P, 512], F32R, name="fc2c", tag="fc2c")
                        nc.sync.dma_start(fc2c[:], fc2_d[kc * P:(kc + 1) * P, n2 * 512:(n2 + 1) * 512])
                        nc.tensor.matmul(out=yp[:], lhsT=gT[:, kc, :],
                                         rhs=fc2c[:], start=(kc == 0), stop=False)
                    nc.tensor.matmul(out=yp[:], lhsT=ones_r1[:],
                                     rhs=brow_fc2[:, n2 * 512:(n2 + 1) * 512],
                                     start=False, stop=True)
                    nc.scalar.copy(y_sb[:, n2 * 512:(n2 + 1) * 512], yp[:])
                l3g_bc = ln_bc("3g")
                l3b_bc = ln_bc("3b")
                out_sb = sbG.tile([P, E], F32, name="out_sb")
                _ln_tail(nc, sbG, y_sb[:], x2[:], l3g_bc[:], l3b_bc[:], out_sb, "ln3")
                nc.sync.dma_start(out_d[:], out_sb[:])
    return nc


_BUILT = {}


def get_built(cfg_key="hw"):
    if cfg_key not in _BUILT:
        cfg = dict(CFG)
        if cfg_key == "sim":
            cfg["use_gelu"] = False
            cfg["MS"] = 1024
        elif cfg_key == "hwsim":
            cfg["use_gelu"] = False
        nc = bacc.Bacc("TRN2", target_bir_lowering=False, debug=False, num_devices=NCORES)
        build(nc, cfg)
        nc.compile()
        _BUILT[cfg_key] = nc
    return _BUILT[cfg_key]


def make_in_maps(inputs, MS=M // NCORES):
    x = np.ascontiguousarray(inputs["x"], dtype=np.float32)
    mk = np.ascontiguousarray(inputs["mem_keys"], dtype=np.float32)
    mv = np.ascontiguousarray(inputs["mem_vals"], dtype=np.float32)
    mkn = mk / np.maximum(np.linalg.norm(mk, axis=-1, keepdims=True), 1e-12)
    in_maps = []
    for c in range(NCORES):
        b, h = c // 2, c % 2
        im = {
            "xb": x[b],
            "xq": x[b, h * P:(h + 1) * P],
            "mkT": np.ascontiguousarray(mkn[c * MS:(c + 1) * MS].T),
            "mv": mv,
            "coreid": np.array([[c]], dtype=np.uint32),
        }
        for pre, names in [("sa", ["q", "k", "v", "o"]), ("ma", ["q", "k", "v", "o"])]:
            for n in names:
                im[f"w_{pre}_{n}"] = np.ascontiguousarray(inputs[f"{pre}_w{n}"], np.float32)
                im[f"b_{pre}_{n}"] = np.ascontiguousarray(inputs[f"{pre}_b{n}"], np.float32).reshape(1, E)
        im["w_fc1"] = np.ascontiguousarray(inputs["fc1_w"], np.float32)
        im["b_fc1"] = np.ascontiguousarray(inputs["fc1_b"], np.float32).reshape(1, FF)
        im["w_fc2"] = np.ascontiguousarray(inputs["fc2_w"], np.float32)
        im["b_fc2"] = np.ascontiguousarray(inputs["fc2_b"], np.float32).reshape(1, E)
        for i, nm in [(1, "1"), (2, "2"), (3, "3")]:
            im[f"ln_{nm}g"] = np.ascontiguousarray(inputs[f"ln{i}_g"], np.float32).reshape(1, E)
            im[f"ln_{nm}b"] = np.ascontiguousarray(inputs[f"ln{i}_b"], np.float32).reshape(1, E)
        in_maps.append(im)
    return in_maps


def assemble(results):
    out = np.zeros((B, S, E), dtype=np.float32)
    for c in range(NCORES):
        b, h = c // 2, c % 2
        out[b, h * P:(h + 1) * P] = results[c]["out"]
    return out


def kernel(**inputs) -> np.ndarray:
    nc = get_built("hw")
    in_maps = make_in_maps(inputs)
    res = bass_utils.run_bass_kernel_spmd(nc, in_maps, core_ids=list(range(NCORES)), trace=False)
    return assemble(res.results)


def kernel_traced(**inputs):
    """Like kernel() but with NTFF tracing; returns (output, BassKernelResults).
    Falls back to untraced when the axon NTFF hook is unavailable."""
    nc = get_built("hw")
    in_maps = make_in_maps(inputs)
    try:
        res = bass_utils.run_bass_kernel_spmd(
            nc, in_maps, core_ids=list(range(NCORES)), trace=True)
    except ModuleNotFoundError:
        res = bass_utils.run_bass_kernel_spmd(
            nc, in_maps, core_ids=list(range(NCORES)), trace=False)
    return assemble(res.results), res

